# revision 16
# baseline (speedup 1.0000x reference)
"""Multi-head self-attention (RoPE, causal) Bass kernel for 8 TRN2 NeuronCores.

Problem: x (2, 2048, 1024) f32, wqkv (3072, 1024), wo (1024, 1024).
  qkv = x @ wqkv.T ; RoPE(q, k) ; causal softmax attention (16 heads, hd=64);
  out = y @ wo.T.

Sharding: batch (2-way) x head-group (4-way) tensor parallel = 8 cores.
Each core computes a full (2048, 1024) partial output for its batch from its
4 heads; host sums the 4 partials per batch (the TP all-reduce done at
unshard time).

v2 design notes (vs the 199us baseline):
- All PE matmuls in f16/bf16: f32r moving streams measured ~2cyc/row on HW
  and f32r LDWEIGHTS is 2-3x slower; f16 runs at 1cyc/row (216ns/512col).
- Pre-tiled DRAM layouts so each logical load is ONE DMA descriptor
  (sync-seq DIRECT2D costs ~605ns each; baseline burned 33us there and
  stalled the PE for ~20us at startup).
- Separate PSUM pools: attention scores (2x full-bank), y accum (2 banks),
  and a half-bank filler pool (bufs=2) for qkv/v/wo units, so the Tile
  list-scheduler can interleave qkv(j+1)/wo(j-1) matmuls into the
  EXP-gated gaps of attention(j) (also keeps the PE p-state at 2.4GHz).
- Engine rebalance: EXP (the hard ~70us ACT wall) + v copies on ACT;
  psum-reading copies/muls on DVE; rope mul/add on GpSimd; y_all->yt
  transposes on the DMA XBAR (no PE transposes); OUT stored f16.
"""
import sys

sys.path.insert(0, "/opt/trn_rl_repo")

import numpy as np

import concourse.bass as bass
import concourse.mybir as mybir
import concourse.tile as tile
from concourse import bacc, bass_utils
from concourse.masks import make_identity

B, L, D = 2, 2048, 1024
NH, HD = 16, 64
NCORES = 8
HPC = 4            # heads per core
LQB = 512          # Lq block per S^T unit
NLQ = L // LQB     # 4
NLT = L // 128     # 16
KT = D // 128      # 8 contraction tiles for projections

F32 = mybir.dt.float32
F32R = mybir.dt.float32r
F16 = mybir.dt.float16
BF16 = mybir.dt.bfloat16

_cache = {}


def build_nc(debug=False):
    nc = bacc.Bacc("TRN2", target_bir_lowering=False, debug=False)

    # pre-tiled DRAM layouts: one DMA descriptor per logical load
    XT3 = nc.dram_tensor("XT3", [128, KT, L], F16, kind="ExternalInput")
    WQKT = nc.dram_tensor("WQKT", [128, KT * 512], F16, kind="ExternalInput")
    WVT = nc.dram_tensor("WVT", [128, KT * 260], F16, kind="ExternalInput")
    WOT = nc.dram_tensor("WOT", [128, 2 * D], F16, kind="ExternalInput")
    PERM = nc.dram_tensor("PERM", [128, 128], F16, kind="ExternalInput")
    COS = nc.dram_tensor("COS", [128, L], F16, kind="ExternalInput")
    SIN = nc.dram_tensor("SIN", [128, L], F16, kind="ExternalInput")
    MASKS = nc.dram_tensor("MASKS", [128, 384], F16, kind="ExternalInput")
    OUT = nc.dram_tensor("OUT", [L, D], F16, kind="ExternalOutput")
    if debug:
        DROT = nc.dram_tensor("DROT", [512, L], F16, kind="ExternalOutput")
        DV = nc.dram_tensor("DV", [NLT * 128, 260], F16, kind="ExternalOutput")
        DY = nc.dram_tensor("DY", [NLT * 128, 256], F16, kind="ExternalOutput")
        DYT = nc.dram_tensor("DYT", [256, L], F16, kind="ExternalOutput")

    with tile.TileContext(nc) as tc:
        with (
            tc.tile_pool(name="consts", bufs=1) as cpool,
            tc.tile_pool(name="weights", bufs=1) as wpool,
            tc.tile_pool(name="qkrot", bufs=1) as rotpool,
            tc.tile_pool(name="vsb", bufs=1) as vpool,
            tc.tile_pool(name="yall", bufs=1) as ypool,
            tc.tile_pool(name="ytr", bufs=1) as ytpool,
            tc.tile_pool(name="xt", bufs=1) as xpool,
            tc.tile_pool(name="raws", bufs=3) as rawpool,
            tc.tile_pool(name="t1s", bufs=3) as t1pool,
            tc.tile_pool(name="t2s", bufs=3) as t2pool,
            tc.tile_pool(name="pts", bufs=6) as ptpool,
            tc.tile_pool(name="outsb", bufs=3) as opool,
            tc.tile_pool(name="recs", bufs=4) as recpool,
            tc.tile_pool(name="psS", bufs=2, space="PSUM") as spool,
            tc.tile_pool(name="psF", bufs=2, space="PSUM") as fpool,
            tc.tile_pool(name="psY", bufs=1, space="PSUM") as psypool,
        ):
            # ---- preamble loads, ordered by first use. wqk+xt0 are
            # chunked/interleaved so the first qkv matmul can start after
            # ~0.5MB instead of the full 2MB.
            wqk_sb = wpool.tile([128, KT * 512], F16, tag="wqk", name="wqk")
            xt_sb = [xpool.tile([128, KT * 512], F16, tag=f"xt{j}",
                                name=f"xt{j}") for j in range(NLQ)]

            def load_xt(j):
                nc.sync.dma_start(xt_sb[j][:], XT3[:, :, j * LQB:(j + 1) * LQB])

            for kc, kw in ((0, 1), (1, 1), (2, 2), (4, 2), (6, 2)):
                ksl = slice(kc * 512, (kc + kw) * 512)
                nc.sync.dma_start(wqk_sb[:, ksl], WQKT[:, ksl])
                nc.sync.dma_start(xt_sb[0][:, ksl],
                                  XT3[:, kc:kc + kw, 0:LQB])

            cos_sb = [cpool.tile([128, LQB], F16, tag=f"cos{j}", name=f"cos{j}")
                      for j in range(NLQ)]
            sin_sb = [cpool.tile([128, LQB], F16, tag=f"sin{j}", name=f"sin{j}")
                      for j in range(NLQ)]

            def load_cs(j):
                xs = slice(j * LQB, (j + 1) * LQB)
                nc.sync.dma_start(cos_sb[j][:], COS[:, xs])
                nc.sync.dma_start(sin_sb[j][:], SIN[:, xs])

            load_cs(0)

            wvt_sb = wpool.tile([128, KT * 260], F16, tag="wvt", name="wvt")
            nc.sync.dma_start(wvt_sb[:], WVT[:, :])
            perm_sb = cpool.tile([128, 128], F16, tag="perm", name="perm")
            nc.sync.dma_start(perm_sb[:], PERM[:, :])
            masks_sb = cpool.tile([128, 384], F16, tag="masks", name="masks")
            nc.sync.dma_start(masks_sb[:], MASKS[:, :])
            negi_sb = masks_sb[:, 0:128]
            bigm2_sb = masks_sb[:, 128:384]
            wot_sb = wpool.tile([128, 2 * D], F16, tag="wot", name="wot")
            idn_sb = cpool.tile([128, 128], F16, tag="idn", name="idn")
            make_identity(nc, idn_sb[:])

            # persistent activation storage
            # qk_rot[m]: m=0,1 -> q head-pairs (h01, h23); m=2,3 -> k pairs
            qk_rot = [rotpool.tile([128, L], F16, tag=f"rot{m}", name=f"rot{m}")
                      for m in range(4)]
            v_sb = [vpool.tile([128, 260], F16, tag=f"v{t}", name=f"v{t}")
                    for t in range(NLT)]
            y_all = [ypool.tile([128, HPC * HD], F16, tag=f"y{i}", name=f"y{i}")
                     for i in range(NLT)]
            yt_sb = [ytpool.tile([128, L], F16, tag=f"yt{c2}", name=f"yt{c2}")
                     for c2 in range(2)]

            def qkv_chunk(j):
                xt = xt_sb[j]
                xs = slice(j * LQB, (j + 1) * LQB)
                # q/k head-pair tiles with rope. For j=0 the xt chunks are
                # still streaming in, so go k-outer/m-inner across two
                # score-pool tiles (idle until attention starts): all four
                # m-units then finish with the last chunk instead of
                # serially after it.
                if j == 0:
                    big = [spool.tile([128, 1024], F32, tag="sp",
                                      name=f"qkv0_{b}") for b in range(2)]
                    ps4 = [big[m // 2][:, 512 * (m % 2):512 * (m % 2) + 512]
                           for m in range(4)]
                    for k in range(KT):
                        for m in range(4):
                            nc.tensor.matmul(
                                ps4[m],
                                wqk_sb[:, k * 512 + m * 128:
                                       k * 512 + (m + 1) * 128],
                                xt[:, k * 512:(k + 1) * 512],
                                start=(k == 0), stop=(k == KT - 1),
                            )
                for m in range(4):
                    psf = None
                    if j == 0:
                        ps = ps4[m]
                    else:
                        psf = fpool.tile([128, 512], F32, tag="f", name="ps")
                        for k in range(KT):
                            nc.tensor.matmul(
                                psf[:],
                                wqk_sb[:, k * 512 + m * 128:
                                       k * 512 + (m + 1) * 128],
                                xt[:, k * 512:(k + 1) * 512],
                                start=(k == 0), stop=(k == KT - 1),
                            )
                        ps = psf[:]
                    raw = rawpool.tile([128, LQB], F16, tag="raw", name="raw")
                    nc.vector.tensor_copy(raw[:], ps)
                    # j>0: reuse the ps bank for the rotate-half product (raw
                    # extracted, so the start=True re-zero is safe - halves
                    # the unit's PSUM footprint). j=0: ps lives in the score
                    # pool, which attention needs back, so use fpool.
                    if psf is None:
                        psw = fpool.tile([128, 512], F32, tag="f", name="psw")
                    else:
                        psw = psf
                    nc.tensor.matmul(psw[:], perm_sb[:], raw[:],
                                     start=True, stop=True)
                    t1 = t1pool.tile([128, LQB], F16, tag="t1", name="t1")
                    nc.gpsimd.tensor_mul(t1[:], raw[:], cos_sb[j][:])
                    t2 = t2pool.tile([128, LQB], F32, tag="t2", name="t2")
                    nc.vector.tensor_mul(t2[:], psw[:], sin_sb[j][:])
                    nc.gpsimd.tensor_add(qk_rot[m][:, xs], t1[:], t2[:])
                # v tiles (natural L x hd layout, ones col after each head)
                for i2 in range(4):
                    ti = j * 4 + i2
                    psv = fpool.tile([128, 512], F32, tag="f", name="psv")
                    for k in range(KT):
                        nc.tensor.matmul(
                            psv[:, 0:260],
                            xt[:, k * 512 + i2 * 128:k * 512 + i2 * 128 + 128],
                            wvt_sb[:, k * 260:(k + 1) * 260],
                            start=(k == 0), stop=(k == KT - 1),
                        )
                    nc.scalar.copy(v_sb[ti][:], psv[:, 0:260])
                    nc.gpsimd.memset(v_sb[ti][:, 64:260:65], 1.0)

            def attention_jq(jq):
                nt = 4 * jq + 4  # causal: Lk tiles 0 .. 4jq+3
                for hp in range(2):
                    y_ps = [
                        psypool.tile([128, 260], F32, tag=f"yps{h}",
                                     name=f"yps{h}", bufs=1)
                        for h in range(2)
                    ]
                    # the first P@V matmul per y_ps bank (t=0, js=0) uses
                    # start=True: it zeroes the whole bank, so the sibling js
                    # regions start from zero without a DVE memset (and
                    # without the DVE->PE dependency at each strip start).
                    for t in range(nt):
                        ks = slice(t * 128, (t + 1) * 128)
                        diag = t >= 4 * jq
                        # causal trim: cols < off are fully masked
                        off = max(0, t * 128 - jq * LQB)
                        r = off // 128
                        sp = spool.tile([128, 1024], F32, tag="sp", name="sp")
                        for h in range(2):
                            hs = slice(64 * h, 64 * h + 64)
                            nc.tensor.matmul(
                                sp[:, 512 * h + off:512 * h + 512],
                                qk_rot[2 + hp][hs, ks],
                                qk_rot[hp][hs, jq * LQB + off:
                                           (jq + 1) * LQB],
                                start=True, stop=not diag,
                            )
                        if diag:
                            # upper-triangle mask via PE accumulate: on the
                            # PE queue right behind the score matmul, so no
                            # cross-engine latency on the exp chain (a DVE
                            # mask here measured +35us end-to-end). One
                            # 256-col stream writes both heads' diag blocks
                            # through a strided psum view.
                            sp3 = sp[:].rearrange("p (b c) -> p b c", b=2)
                            nc.tensor.matmul(
                                sp3[:, :, off:off + 128],
                                negi_sb[:],
                                bigm2_sb[:],
                                start=False, stop=True,
                                skip_group_check=True,
                            )
                        pt = ptpool.tile([128, 1024], F16, tag="pt", name="pt")
                        nc.scalar.activation(
                            pt[:, off:1024], sp[:, off:1024],
                            mybir.ActivationFunctionType.Exp
                        )
                        for h in range(2):
                            H = 2 * hp + h
                            for js in range(r, 4):
                                nc.tensor.matmul(
                                    y_ps[h][:, 65 * js:65 * js + 65],
                                    pt[:, 512 * h + 128 * js:
                                       512 * h + 128 * js + 128],
                                    v_sb[t][:, 65 * H:65 * H + 65],
                                    start=(t == 0 and js == 0),
                                    stop=(t == nt - 1),
                                    skip_group_check=True,
                                )
                    # normalize: y /= rowsum, write into y_all as fp16,
                    # js-major so each i-tile finishes both heads early and
                    # its yt transpose (c2 == hp, DMA XBAR) can issue while
                    # the remaining norms run. On the last strip ACT is idle
                    # (no more EXPs), so alternate muls between DVE and ACT.
                    last = jq == NLQ - 1
                    recs = []
                    for h in range(2):
                        rec = recpool.tile([128, 4], F32, tag=f"rec{h}",
                                           name="rec")
                        nc.vector.reciprocal(rec[:], y_ps[h][:, 64:260:65])
                        recs.append(rec)
                    for js in range(4):
                        i = 4 * jq + js
                        for h in range(2):
                            H = 2 * hp + h
                            dst = y_all[i][:, HD * H:HD * H + HD]
                            srcp = y_ps[h][:, 65 * js:65 * js + 64]
                            if last and h == 0:
                                nc.scalar.mul(dst, srcp,
                                              recs[h][:, js:js + 1])
                            else:
                                nc.vector.tensor_scalar_mul(
                                    dst, srcp, recs[h][:, js:js + 1])
                        nc.sync.dma_start(
                            yt_sb[hp][:, 128 * i:128 * i + 128],
                            y_all[i][:, 128 * hp:128 * hp + 128],
                            transpose=True,
                        )

            def wo_tiles(jq):
                last = jq == NLQ - 1
                for i in range(4 * jq, 4 * jq + 4):
                    ob = opool.tile([128, 1024], F16, tag="ob", name="ob")
                    for half in range(2):
                        po = fpool.tile([128, 512], F32, tag="f", name="po")
                        for c2 in range(2):
                            nc.tensor.matmul(
                                po[:],
                                yt_sb[c2][:, 128 * i:128 * i + 128],
                                wot_sb[:, c2 * D + 512 * half:
                                       c2 * D + 512 * half + 512],
                                start=(c2 == 0), stop=(c2 == 1),
                            )
                        obs = ob[:, 512 * half:512 * half + 512]
                        if last:
                            if half == 0:
                                nc.scalar.copy(obs, po[:])
                            else:
                                nc.vector.tensor_copy(obs, po[:])
                        else:
                            nc.vector.tensor_copy(obs, po[:])
                    nc.sync.dma_start(OUT[128 * i:128 * i + 128, :], ob[:])

            # defer each chunk's wo by one iteration: its inputs are long
            # ready, so the scheduler uses it to fill the PE bubble at the
            # attention(j) tail / qkv(j+1) seam
            for j in range(NLQ):
                qkv_chunk(j)
                if j + 1 < NLQ:
                    load_xt(j + 1)
                    load_cs(j + 1)
                if j == 0:
                    nc.sync.dma_start(wot_sb[:], WOT[:, :])
                if j > 0:
                    wo_tiles(j - 1)
                attention_jq(j)
            wo_tiles(NLQ - 1)

            if debug:
                for m in range(4):
                    nc.sync.dma_start(DROT[128 * m:128 * m + 128, :],
                                      qk_rot[m][:])
                for t in range(NLT):
                    nc.sync.dma_start(DV[128 * t:128 * t + 128, :], v_sb[t][:])
                for i in range(NLT):
                    nc.sync.dma_start(DY[128 * i:128 * i + 128, :], y_all[i][:])
                for c2 in range(2):
                    nc.sync.dma_start(DYT[128 * c2:128 * c2 + 128, :],
                                      yt_sb[c2][:])

    nc.finalize()
    return nc


def prep_inputs(x, wqkv, wo):
    """Build the 8 per-core input dicts from the full-problem inputs."""
    import ml_dtypes
    bf = ml_dtypes.bfloat16

    x = np.asarray(x, dtype=np.float32)
    wqkv = np.asarray(wqkv, dtype=np.float32)
    wo = np.asarray(wo, dtype=np.float32)

    # rope tables
    inv_freq = 1.0 / (10000.0 ** (np.arange(0, HD, 2, dtype=np.float32) / HD))
    t = np.arange(L, dtype=np.float32)
    freqs = np.outer(t, inv_freq)                  # (L, 32)
    cos32 = np.cos(freqs).T.astype(np.float32)     # (32, L)
    sin32 = np.sin(freqs).T.astype(np.float32)
    COS = np.ascontiguousarray(np.tile(cos32, (4, 1)))           # (128, L)
    SIN = np.ascontiguousarray(
        np.concatenate([-sin32, sin32, -sin32, sin32], axis=0)
    )

    # 32-block swap permutation (within each head's 64 rows)
    PERM = np.zeros((128, 128), dtype=np.float32)
    for blk in range(2):
        o = 64 * blk
        PERM[o:o + 32, o + 32:o + 64] = np.eye(32)
        PERM[o + 32:o + 64, o:o + 32] = np.eye(32)

    NEGI = (-30000.0 * np.eye(128)).astype(np.float16)
    BIGM = (np.arange(128)[None, :] < np.arange(128)[:, None]).astype(np.float16)
    MASKS = np.ascontiguousarray(np.concatenate([NEGI, BIGM, BIGM], axis=1))

    in_maps = []
    scale = np.float32(HD ** -0.5)
    for c in range(NCORES):
        b, g = divmod(c, 4)
        qrows = slice(256 * g, 256 * g + 256)
        krows = slice(1024 + 256 * g, 1024 + 256 * g + 256)
        vrows = slice(2048 + 256 * g, 2048 + 256 * g + 256)

        xT = np.ascontiguousarray(x[b].T)                        # (1024, 2048)
        # XT3[p, k, l] = xT[k*128+p, l]
        XT3 = np.ascontiguousarray(
            xT.reshape(KT, 128, L).transpose(1, 0, 2))           # (128, 8, L)
        wq = (wqkv[qrows, :] * scale).T                          # (1024, 256)
        wk = wqkv[krows, :].T
        wqkT = np.concatenate([wq, wk], axis=1)                  # (1024, 512)
        # WQKT[p, k*512 + c] = wqkT[k*128+p, c]
        WQKT = np.ascontiguousarray(
            wqkT.reshape(KT, 128, 512).transpose(1, 0, 2).reshape(128, KT * 512))
        vpart = wqkv[vrows, :].T                                 # (1024, 256)
        wvT = np.zeros((D, 260), dtype=np.float32)
        for h in range(HPC):
            wvT[:, 65 * h:65 * h + 64] = vpart[:, 64 * h:64 * h + 64]
        WVT = np.ascontiguousarray(
            wvT.reshape(KT, 128, 260).transpose(1, 0, 2).reshape(128, KT * 260))
        woT = np.ascontiguousarray(wo[:, 256 * g:256 * g + 256].T)  # (256, 1024)
        WOT = np.ascontiguousarray(
            woT.reshape(2, 128, D).transpose(1, 0, 2).reshape(128, 2 * D))

        in_maps.append({
            "XT3": XT3.astype(np.float16),
            "WQKT": WQKT.astype(np.float16),
            "WVT": WVT.astype(np.float16),
            "WOT": WOT.astype(np.float16),
            "COS": COS.astype(np.float16),
            "SIN": SIN.astype(np.float16),
            "PERM": PERM.astype(np.float16),
            "MASKS": MASKS,
        })
    return in_maps


def kernel(x, wqkv, wo):
    if "nc" not in _cache:
        _cache["nc"] = build_nc()
    nc = _cache["nc"]
    in_maps = prep_inputs(x, wqkv, wo)
    res = bass_utils.run_bass_kernel_spmd(nc, in_maps, list(range(NCORES)))
    outs = [res.results[c]["OUT"].astype(np.float32) for c in range(NCORES)]
    out0 = outs[0] + outs[1] + outs[2] + outs[3]
    out1 = outs[4] + outs[5] + outs[6] + outs[7]
    return np.stack([out0, out1]).astype(np.float32)


# revision 17
# speedup vs baseline: 1.0396x; 1.0396x over previous
"""Multi-head self-attention (RoPE, causal) Bass kernel for 8 TRN2 NeuronCores.

Problem: x (2, 2048, 1024) f32, wqkv (3072, 1024), wo (1024, 1024).
  qkv = x @ wqkv.T ; RoPE(q, k) ; causal softmax attention (16 heads, hd=64);
  out = y @ wo.T.

Sharding: batch (2-way) x head-group (4-way) tensor parallel = 8 cores.
Each core computes a full (2048, 1024) partial output for its batch from its
4 heads; host sums the 4 partials per batch (the TP all-reduce done at
unshard time).

v2 design notes (vs the 199us baseline):
- All PE matmuls in f16/bf16: f32r moving streams measured ~2cyc/row on HW
  and f32r LDWEIGHTS is 2-3x slower; f16 runs at 1cyc/row (216ns/512col).
- Pre-tiled DRAM layouts so each logical load is ONE DMA descriptor
  (sync-seq DIRECT2D costs ~605ns each; baseline burned 33us there and
  stalled the PE for ~20us at startup).
- Separate PSUM pools: attention scores (2x full-bank), y accum (2 banks),
  and a half-bank filler pool (bufs=2) for qkv/v/wo units, so the Tile
  list-scheduler can interleave qkv(j+1)/wo(j-1) matmuls into the
  EXP-gated gaps of attention(j) (also keeps the PE p-state at 2.4GHz).
- Engine rebalance: EXP (the hard ~70us ACT wall) + v copies on ACT;
  psum-reading copies/muls on DVE; rope mul/add on GpSimd; y_all->yt
  transposes on the DMA XBAR (no PE transposes); OUT stored f16.
"""
import sys

sys.path.insert(0, "/opt/trn_rl_repo")

import numpy as np

import concourse.bass as bass
import concourse.mybir as mybir
import concourse.tile as tile
from concourse import bacc, bass_utils
from concourse.masks import make_identity

B, L, D = 2, 2048, 1024
NH, HD = 16, 64
NCORES = 8
HPC = 4            # heads per core
LQB = 512          # Lq block per S^T unit
NLQ = L // LQB     # 4
NLT = L // 128     # 16
KT = D // 128      # 8 contraction tiles for projections

F32 = mybir.dt.float32
F32R = mybir.dt.float32r
F16 = mybir.dt.float16
BF16 = mybir.dt.bfloat16

_cache = {}


def build_nc(debug=False):
    nc = bacc.Bacc("TRN2", target_bir_lowering=False, debug=False)

    # pre-tiled DRAM layouts: one DMA descriptor per logical load
    XT3 = nc.dram_tensor("XT3", [128, KT, L], F16, kind="ExternalInput")
    WQKT = nc.dram_tensor("WQKT", [128, KT * 512], F16, kind="ExternalInput")
    WVT = nc.dram_tensor("WVT", [128, KT * 260], F16, kind="ExternalInput")
    WOT = nc.dram_tensor("WOT", [128, 2 * D], F16, kind="ExternalInput")
    PERM = nc.dram_tensor("PERM", [128, 128], F16, kind="ExternalInput")
    COS = nc.dram_tensor("COS", [128, L], F16, kind="ExternalInput")
    SIN = nc.dram_tensor("SIN", [128, L], F16, kind="ExternalInput")
    MASKS = nc.dram_tensor("MASKS", [128, 384], F16, kind="ExternalInput")
    OUT = nc.dram_tensor("OUT", [L, D], F16, kind="ExternalOutput")
    if debug:
        DROT = nc.dram_tensor("DROT", [512, L], F16, kind="ExternalOutput")
        DV = nc.dram_tensor("DV", [NLT * 128, 260], F16, kind="ExternalOutput")
        DY = nc.dram_tensor("DY", [NLT * 128, 256], F16, kind="ExternalOutput")
        DYT = nc.dram_tensor("DYT", [256, L], F16, kind="ExternalOutput")

    with tile.TileContext(nc) as tc:
        with (
            tc.tile_pool(name="consts", bufs=1) as cpool,
            tc.tile_pool(name="weights", bufs=1) as wpool,
            tc.tile_pool(name="qkrot", bufs=1) as rotpool,
            tc.tile_pool(name="vsb", bufs=1) as vpool,
            tc.tile_pool(name="yall", bufs=1) as ypool,
            tc.tile_pool(name="ytr", bufs=1) as ytpool,
            tc.tile_pool(name="xt", bufs=1) as xpool,
            tc.tile_pool(name="raws", bufs=3) as rawpool,
            tc.tile_pool(name="t1s", bufs=3) as t1pool,
            tc.tile_pool(name="t2s", bufs=3) as t2pool,
            tc.tile_pool(name="pts", bufs=6) as ptpool,
            tc.tile_pool(name="outsb", bufs=3) as opool,
            tc.tile_pool(name="recs", bufs=4) as recpool,
            tc.tile_pool(name="psS", bufs=2, space="PSUM") as spool,
            tc.tile_pool(name="psF", bufs=2, space="PSUM") as fpool,
            tc.tile_pool(name="psY", bufs=1, space="PSUM") as psypool,
        ):
            # ---- preamble loads, ordered by first use. wqk+xt0 are
            # chunked/interleaved so the first qkv matmul can start after
            # ~0.5MB instead of the full 2MB.
            wqk_sb = wpool.tile([128, KT * 512], F16, tag="wqk", name="wqk")
            xt_sb = [xpool.tile([128, KT * 512], F16, tag=f"xt{j}",
                                name=f"xt{j}") for j in range(NLQ)]

            def load_xt(j):
                nc.sync.dma_start(xt_sb[j][:], XT3[:, :, j * LQB:(j + 1) * LQB])

            for kc, kw in ((0, 1), (1, 1), (2, 2), (4, 2), (6, 2)):
                ksl = slice(kc * 512, (kc + kw) * 512)
                nc.sync.dma_start(wqk_sb[:, ksl], WQKT[:, ksl])
                nc.sync.dma_start(xt_sb[0][:, ksl],
                                  XT3[:, kc:kc + kw, 0:LQB])

            cos_sb = [cpool.tile([128, LQB], F16, tag=f"cos{j}", name=f"cos{j}")
                      for j in range(NLQ)]
            sin_sb = [cpool.tile([128, LQB], F16, tag=f"sin{j}", name=f"sin{j}")
                      for j in range(NLQ)]

            def load_cs(j):
                xs = slice(j * LQB, (j + 1) * LQB)
                nc.sync.dma_start(cos_sb[j][:], COS[:, xs])
                nc.sync.dma_start(sin_sb[j][:], SIN[:, xs])

            load_cs(0)

            wvt_sb = wpool.tile([128, KT * 260], F16, tag="wvt", name="wvt")
            nc.sync.dma_start(wvt_sb[:], WVT[:, :])
            perm_sb = cpool.tile([128, 128], F16, tag="perm", name="perm")
            nc.sync.dma_start(perm_sb[:], PERM[:, :])
            masks_sb = cpool.tile([128, 384], F16, tag="masks", name="masks")
            nc.sync.dma_start(masks_sb[:], MASKS[:, :])
            negi_sb = masks_sb[:, 0:128]
            bigm2_sb = masks_sb[:, 128:384]
            wot_sb = wpool.tile([128, 2 * D], F16, tag="wot", name="wot")
            idn_sb = cpool.tile([128, 128], F16, tag="idn", name="idn")
            make_identity(nc, idn_sb[:])

            # persistent activation storage
            # qk_rot[m]: m=0,1 -> q head-pairs (h01, h23); m=2,3 -> k pairs
            qk_rot = [rotpool.tile([128, L], F16, tag=f"rot{m}", name=f"rot{m}")
                      for m in range(4)]
            v_sb = [vpool.tile([128, 260], F16, tag=f"v{t}", name=f"v{t}")
                    for t in range(NLT)]
            y_all = [ypool.tile([128, HPC * HD], F16, tag=f"y{i}", name=f"y{i}")
                     for i in range(NLT)]
            yt_sb = [ytpool.tile([128, L], F16, tag=f"yt{c2}", name=f"yt{c2}")
                     for c2 in range(2)]

            def qkv_chunk(j):
                xt = xt_sb[j]
                xs = slice(j * LQB, (j + 1) * LQB)
                for m in range(4):
                    psf = fpool.tile([128, 512], F32, tag="f", name="ps")
                    for k in range(KT):
                        nc.tensor.matmul(
                            psf[:],
                            wqk_sb[:, k * 512 + m * 128:
                                   k * 512 + (m + 1) * 128],
                            xt[:, k * 512:(k + 1) * 512],
                            start=(k == 0), stop=(k == KT - 1),
                        )
                    raw = rawpool.tile([128, LQB], F16, tag="raw", name="raw")
                    # j=0: ACT is idle until attention starts - take the raw
                    # extraction (and the f16 t1 mul on DVE's 2x mode) off
                    # the Pool/DVE rope chain to shorten time-to-attention.
                    if j == 0:
                        nc.scalar.copy(raw[:], psf[:])
                    else:
                        nc.vector.tensor_copy(raw[:], psf[:])
                    # reuse the ps bank for the rotate-half product: raw has
                    # been extracted, so the start=True re-zero is safe and
                    # halves this unit's PSUM footprint.
                    psw = psf
                    nc.tensor.matmul(psw[:], perm_sb[:], raw[:],
                                     start=True, stop=True)
                    t1 = t1pool.tile([128, LQB], F16, tag="t1", name="t1")
                    if j == 0:
                        nc.vector.tensor_mul(t1[:], raw[:], cos_sb[j][:])
                    else:
                        nc.gpsimd.tensor_mul(t1[:], raw[:], cos_sb[j][:])
                    t2 = t2pool.tile([128, LQB], F32, tag="t2", name="t2")
                    nc.vector.tensor_mul(t2[:], psw[:], sin_sb[j][:])
                    nc.gpsimd.tensor_add(qk_rot[m][:, xs], t1[:], t2[:])
                # v tiles (natural L x hd layout, ones col after each head)
                for i2 in range(4):
                    ti = j * 4 + i2
                    psv = fpool.tile([128, 512], F32, tag="f", name="psv")
                    for k in range(KT):
                        nc.tensor.matmul(
                            psv[:, 0:260],
                            xt[:, k * 512 + i2 * 128:k * 512 + i2 * 128 + 128],
                            wvt_sb[:, k * 260:(k + 1) * 260],
                            start=(k == 0), stop=(k == KT - 1),
                        )
                    nc.scalar.copy(v_sb[ti][:], psv[:, 0:260])
                    nc.gpsimd.memset(v_sb[ti][:, 64:260:65], 1.0)

            def attention_jq(jq):
                nt = 4 * jq + 4  # causal: Lk tiles 0 .. 4jq+3
                for hp in range(2):
                    y_ps = [
                        psypool.tile([128, 260], F32, tag=f"yps{h}",
                                     name=f"yps{h}", bufs=1)
                        for h in range(2)
                    ]
                    # the first P@V matmul per y_ps bank (t=0, js=0) uses
                    # start=True: it zeroes the whole bank, so the sibling js
                    # regions start from zero without a DVE memset (and
                    # without the DVE->PE dependency at each strip start).
                    for t in range(nt):
                        ks = slice(t * 128, (t + 1) * 128)
                        diag = t >= 4 * jq
                        # causal trim: cols < off are fully masked
                        off = max(0, t * 128 - jq * LQB)
                        r = off // 128
                        sp = spool.tile([128, 1024], F32, tag="sp", name="sp")
                        for h in range(2):
                            hs = slice(64 * h, 64 * h + 64)
                            nc.tensor.matmul(
                                sp[:, 512 * h + off:512 * h + 512],
                                qk_rot[2 + hp][hs, ks],
                                qk_rot[hp][hs, jq * LQB + off:
                                           (jq + 1) * LQB],
                                start=True, stop=not diag,
                            )
                        if diag:
                            # upper-triangle mask via PE accumulate: on the
                            # PE queue right behind the score matmul, so no
                            # cross-engine latency on the exp chain (a DVE
                            # mask here measured +35us end-to-end). One
                            # 256-col stream writes both heads' diag blocks
                            # through a strided psum view.
                            sp3 = sp[:].rearrange("p (b c) -> p b c", b=2)
                            nc.tensor.matmul(
                                sp3[:, :, off:off + 128],
                                negi_sb[:],
                                bigm2_sb[:],
                                start=False, stop=True,
                                skip_group_check=True,
                            )
                        pt = ptpool.tile([128, 1024], F16, tag="pt", name="pt")
                        nc.scalar.activation(
                            pt[:, off:1024], sp[:, off:1024],
                            mybir.ActivationFunctionType.Exp
                        )
                        for h in range(2):
                            H = 2 * hp + h
                            for js in range(r, 4):
                                nc.tensor.matmul(
                                    y_ps[h][:, 65 * js:65 * js + 65],
                                    pt[:, 512 * h + 128 * js:
                                       512 * h + 128 * js + 128],
                                    v_sb[t][:, 65 * H:65 * H + 65],
                                    start=(t == 0 and js == 0),
                                    stop=(t == nt - 1),
                                    skip_group_check=True,
                                )
                    # normalize: y /= rowsum, write into y_all as fp16,
                    # js-major so each i-tile finishes both heads early and
                    # its yt transpose (c2 == hp, DMA XBAR) can issue while
                    # the remaining norms run. On the last strip ACT is idle
                    # (no more EXPs), so alternate muls between DVE and ACT.
                    last = jq == NLQ - 1
                    recs = []
                    for h in range(2):
                        rec = recpool.tile([128, 4], F32, tag=f"rec{h}",
                                           name="rec")
                        nc.vector.reciprocal(rec[:], y_ps[h][:, 64:260:65])
                        recs.append(rec)
                    for js in range(4):
                        i = 4 * jq + js
                        for h in range(2):
                            H = 2 * hp + h
                            dst = y_all[i][:, HD * H:HD * H + HD]
                            srcp = y_ps[h][:, 65 * js:65 * js + 64]
                            if last and h == 0:
                                nc.scalar.mul(dst, srcp,
                                              recs[h][:, js:js + 1])
                            else:
                                nc.vector.tensor_scalar_mul(
                                    dst, srcp, recs[h][:, js:js + 1])
                        nc.sync.dma_start(
                            yt_sb[hp][:, 128 * i:128 * i + 128],
                            y_all[i][:, 128 * hp:128 * hp + 128],
                            transpose=True,
                        )

            def wo_tiles(jq):
                last = jq == NLQ - 1
                for i in range(4 * jq, 4 * jq + 4):
                    ob = opool.tile([128, 1024], F16, tag="ob", name="ob")
                    for half in range(2):
                        po = fpool.tile([128, 512], F32, tag="f", name="po")
                        for c2 in range(2):
                            nc.tensor.matmul(
                                po[:],
                                yt_sb[c2][:, 128 * i:128 * i + 128],
                                wot_sb[:, c2 * D + 512 * half:
                                       c2 * D + 512 * half + 512],
                                start=(c2 == 0), stop=(c2 == 1),
                            )
                        obs = ob[:, 512 * half:512 * half + 512]
                        if last:
                            if half == 0:
                                nc.scalar.copy(obs, po[:])
                            else:
                                nc.vector.tensor_copy(obs, po[:])
                        else:
                            nc.vector.tensor_copy(obs, po[:])
                    nc.sync.dma_start(OUT[128 * i:128 * i + 128, :], ob[:])

            # defer each chunk's wo by one iteration: its inputs are long
            # ready, so the scheduler uses it to fill the PE bubble at the
            # attention(j) tail / qkv(j+1) seam
            for j in range(NLQ):
                qkv_chunk(j)
                if j + 1 < NLQ:
                    load_xt(j + 1)
                    load_cs(j + 1)
                if j == 0:
                    nc.sync.dma_start(wot_sb[:], WOT[:, :])
                if j > 0:
                    wo_tiles(j - 1)
                attention_jq(j)
            wo_tiles(NLQ - 1)

            if debug:
                for m in range(4):
                    nc.sync.dma_start(DROT[128 * m:128 * m + 128, :],
                                      qk_rot[m][:])
                for t in range(NLT):
                    nc.sync.dma_start(DV[128 * t:128 * t + 128, :], v_sb[t][:])
                for i in range(NLT):
                    nc.sync.dma_start(DY[128 * i:128 * i + 128, :], y_all[i][:])
                for c2 in range(2):
                    nc.sync.dma_start(DYT[128 * c2:128 * c2 + 128, :],
                                      yt_sb[c2][:])

    nc.finalize()
    return nc


def prep_inputs(x, wqkv, wo):
    """Build the 8 per-core input dicts from the full-problem inputs."""
    import ml_dtypes
    bf = ml_dtypes.bfloat16

    x = np.asarray(x, dtype=np.float32)
    wqkv = np.asarray(wqkv, dtype=np.float32)
    wo = np.asarray(wo, dtype=np.float32)

    # rope tables
    inv_freq = 1.0 / (10000.0 ** (np.arange(0, HD, 2, dtype=np.float32) / HD))
    t = np.arange(L, dtype=np.float32)
    freqs = np.outer(t, inv_freq)                  # (L, 32)
    cos32 = np.cos(freqs).T.astype(np.float32)     # (32, L)
    sin32 = np.sin(freqs).T.astype(np.float32)
    COS = np.ascontiguousarray(np.tile(cos32, (4, 1)))           # (128, L)
    SIN = np.ascontiguousarray(
        np.concatenate([-sin32, sin32, -sin32, sin32], axis=0)
    )

    # 32-block swap permutation (within each head's 64 rows)
    PERM = np.zeros((128, 128), dtype=np.float32)
    for blk in range(2):
        o = 64 * blk
        PERM[o:o + 32, o + 32:o + 64] = np.eye(32)
        PERM[o + 32:o + 64, o:o + 32] = np.eye(32)

    NEGI = (-30000.0 * np.eye(128)).astype(np.float16)
    BIGM = (np.arange(128)[None, :] < np.arange(128)[:, None]).astype(np.float16)
    MASKS = np.ascontiguousarray(np.concatenate([NEGI, BIGM, BIGM], axis=1))

    in_maps = []
    scale = np.float32(HD ** -0.5)
    for c in range(NCORES):
        b, g = divmod(c, 4)
        qrows = slice(256 * g, 256 * g + 256)
        krows = slice(1024 + 256 * g, 1024 + 256 * g + 256)
        vrows = slice(2048 + 256 * g, 2048 + 256 * g + 256)

        xT = np.ascontiguousarray(x[b].T)                        # (1024, 2048)
        # XT3[p, k, l] = xT[k*128+p, l]
        XT3 = np.ascontiguousarray(
            xT.reshape(KT, 128, L).transpose(1, 0, 2))           # (128, 8, L)
        wq = (wqkv[qrows, :] * scale).T                          # (1024, 256)
        wk = wqkv[krows, :].T
        wqkT = np.concatenate([wq, wk], axis=1)                  # (1024, 512)
        # WQKT[p, k*512 + c] = wqkT[k*128+p, c]
        WQKT = np.ascontiguousarray(
            wqkT.reshape(KT, 128, 512).transpose(1, 0, 2).reshape(128, KT * 512))
        vpart = wqkv[vrows, :].T                                 # (1024, 256)
        wvT = np.zeros((D, 260), dtype=np.float32)
        for h in range(HPC):
            wvT[:, 65 * h:65 * h + 64] = vpart[:, 64 * h:64 * h + 64]
        WVT = np.ascontiguousarray(
            wvT.reshape(KT, 128, 260).transpose(1, 0, 2).reshape(128, KT * 260))
        woT = np.ascontiguousarray(wo[:, 256 * g:256 * g + 256].T)  # (256, 1024)
        WOT = np.ascontiguousarray(
            woT.reshape(2, 128, D).transpose(1, 0, 2).reshape(128, 2 * D))

        in_maps.append({
            "XT3": XT3.astype(np.float16),
            "WQKT": WQKT.astype(np.float16),
            "WVT": WVT.astype(np.float16),
            "WOT": WOT.astype(np.float16),
            "COS": COS.astype(np.float16),
            "SIN": SIN.astype(np.float16),
            "PERM": PERM.astype(np.float16),
            "MASKS": MASKS,
        })
    return in_maps


def kernel(x, wqkv, wo):
    if "nc" not in _cache:
        _cache["nc"] = build_nc()
    nc = _cache["nc"]
    in_maps = prep_inputs(x, wqkv, wo)
    res = bass_utils.run_bass_kernel_spmd(nc, in_maps, list(range(NCORES)))
    outs = [res.results[c]["OUT"].astype(np.float32) for c in range(NCORES)]
    out0 = outs[0] + outs[1] + outs[2] + outs[3]
    out1 = outs[4] + outs[5] + outs[6] + outs[7]
    return np.stack([out0, out1]).astype(np.float32)
